# revision 24
# baseline (speedup 1.0000x reference)
"""Trainium2 Bass kernel for nn_ContextualAttention_25726854103141.

Self-contained: hardcodes shapes B=4,C=128,H=W=64, RATE=2, KSIZE=3.

Distribution: 8 cores = 4 samples x 2 column-halves of the score matrix
(data-parallel over batch + split over the f-pixel axis n). One uniform
SPMD program; per-core behavior differs only through input data.

v4 design (from v2 baseline at 162us; trace-driven):
- DMA-friendly input layouts: bdT split into r-chunks, fs9 into halves,
  small vectors packed into one [128,6] tensor. Every dma_start is
  per-partition contiguous (128 big descriptors instead of thousands of
  256B packets). The two most critical chunks (bdT7, fs9a) are issued
  from the GpSimd queue, whose prologue finishes ~1.7us before Sync's.
- PE warm-up burst during the DMA wait so the scores GEMM starts at the
  full 2.4GHz clock (HAM ramp needs ~3us of continuous PE activity).
- ONE psum pool for the whole kernel with four tag-chains (pd/pu/pss/pe,
  2 banks each). v3's per-phase pools serialized fuse2 behind the entire
  scores+fuse1 phase via the pool alloc boundary; tag reuse makes each
  new tile wait only on the previous user of its own slot.
- zc boundary zeroing folded into the scores PSUM evacuation
  (activation Copy with per-partition scale AP).
- U1/D1 wrap rows evacuated from PSUM to SBUF immediately.
- scores round order [7,0,3,4,1,2,5,6]: fuse1 g02|g24 (DVE|GpSimd in
  parallel) start after r2, g46|g68 after r6; fuse2 then streams with
  at most a ~2us PE bubble.
- fuse1 ranges shrunk to the columns actually read downstream (main
  window [0:672]); the expensive tiny column copies are gone, the
  boundary column fixes moved to the scalar engine.
- softmax reciprocal via reciprocal_approx_fast (5x faster; 18 bits is
  plenty for bf16 downstream); mm/4 folded into the f32->bf16 cast;
  Ssoft muls split DVE/GpSimd; colsums + broadcast keep PE warm.
- output DMA chunked per conv2 row-group (overlaps the conv tail).
"""
import numpy as np

SCALE = 10.0
KSH = 45.0
WM, WA = 704, 64          # main window cols, aux window cols
WTOT = WM + 2 * WA        # 832
NEED_LO, NEED_HI = 64, 640
ND = NEED_HI - NEED_LO    # 576
ME = 672                  # main cols actually read downstream

_CACHE = {}

TAPS9 = [(k, l) for k in range(3) for l in range(3)]


# ----------------------------------------------------------------------
# host-side helpers
# ----------------------------------------------------------------------
def _ds_indices(oh, H):
    j = np.arange(oh, dtype=np.float32)
    g = j / np.float32(oh - 1) * np.float32(2) - np.float32(1)
    ih = np.round(((g + 1) * np.float32(H) - 1) / np.float32(2))
    valid = (ih >= 0) & (ih <= H - 1)
    return np.clip(ih, 0, H - 1).astype(np.int32), valid


def _nearest_ds(x, oh, ow):
    H, W = x.shape[-2], x.shape[-1]
    ih, vh = _ds_indices(oh, H)
    iw, vw = _ds_indices(ow, W)
    out = x[..., ih, :][..., iw]
    return (out * (vh[:, None] & vw[None, :]).astype(x.dtype)).astype(np.float32)


def _mats():
    """[9][128,128] stationary matrices: out[m,n]=sum_k M[k,m]*x[k,n]."""
    ident = np.eye(128, dtype=np.float32)
    s4p = np.zeros((128, 128), np.float32)   # out[m] = in[m+4]
    for m in range(124):
        s4p[m + 4, m] = 1.0
    s4m = np.zeros((128, 128), np.float32)   # out[m] = in[m-4]
    for m in range(4, 128):
        s4m[m - 4, m] = 1.0
    selp = np.zeros((128, 128), np.float32)  # out[124+t] = in[t]
    for t in range(4):
        selp[t, 124 + t] = 1.0
    selp7 = np.zeros((128, 128), np.float32)  # out[124+t] = in[1+t], t<3
    for t in range(3):
        selp7[1 + t, 124 + t] = 1.0
    selm = np.zeros((128, 128), np.float32)  # out[t] = in[124+t]
    for t in range(4):
        selm[124 + t, t] = 1.0
    selm0 = np.zeros((128, 128), np.float32)  # out[1+t] = in[124+t], t<3
    for t in range(3):
        selm0[124 + t, 1 + t] = 1.0
    u1m = np.zeros((128, 128), np.float32)   # out[m] = in[m+1]
    for m in range(127):
        u1m[m + 1, m] = 1.0
    d1m = np.zeros((128, 128), np.float32)   # out[m] = in[m-1]
    for m in range(1, 128):
        d1m[m - 1, m] = 1.0
    return np.stack([ident, s4p, s4m, selp, selp7, selm, selm0, u1m, d1m])


(M_ID, M_S4P, M_S4M, M_SELP, M_SELP7, M_SELM, M_SELM0, M_U1,
 M_D1) = range(9)


def _make_bdT(b_ds):
    """[128, 9, 1024] f32: bdT[c', 3k+l, 128r+q] =
    bdp[q, 4r + c'//32 + k, c'%32 + l] / norm[c']  (bdp = padded b_ds)."""
    bdp = np.pad(b_ds, ((0, 0), (1, 1), (1, 1)))
    W = np.lib.stride_tricks.sliding_window_view(bdp, (3, 3), axis=(1, 2))
    # W[q, h, w, k, l], h/w in 0..31
    A = np.ascontiguousarray(W.reshape(128, 8, 4, 32, 3, 3))
    n2 = (A * A).sum(axis=(0, 1, 4, 5))                    # [hi, wi]
    norm = np.maximum(np.sqrt(n2), 1e-4).astype(np.float32)
    bdT = A.transpose(2, 3, 4, 5, 1, 0).reshape(128, 9, 1024)
    return np.ascontiguousarray(bdT / norm.reshape(128, 1, 1))


def _make_fs9(f_ds, h):
    """[128, 9, 832] f32: per-core shifted f windows (704 main + 2x64 aux)."""
    fsp = np.pad(f_ds, ((0, 0), (1, 1), (1, 1)))   # (128, 34, 34)
    um = -2 if h == 0 else 12
    fdp = np.zeros((128, 24, 34), np.float32)
    for bt in range(24):
        gu = um + bt
        if 0 <= gu < 34:
            fdp[:, bt, :] = fsp[:, gu, :]
    fxm = np.zeros((128, 4, 34), np.float32)
    fxp = np.zeros((128, 4, 34), np.float32)
    if h == 0:
        fxm[:] = fsp[:, 30:34, :]
    else:
        fxp[:] = fsp[:, 0:4, :]
    fs9 = np.zeros((128, 9, WTOT), np.float32)
    for j, (k, l) in enumerate(TAPS9):
        fs9[:, j, 0:WM] = fdp[:, k:k + 22, l:l + 32].reshape(128, WM)
        fs9[:, j, WM:WM + WA] = fxm[:, k:k + 2, l:l + 32].reshape(128, WA)
        fs9[:, j, WM + WA:WTOT] = fxp[:, k:k + 2, l:l + 32].reshape(128, WA)
    return fs9


R_SCORE = [3, 4, 1, 2, 5, 6]            # rounds after r7/r0 (see DMA order)
R_FUSE2 = [5, 6, 4, 3, 2, 1, 0, 7]      # rounds gated on earliest S1 rows


# ----------------------------------------------------------------------
# device program (uniform across cores)
# ----------------------------------------------------------------------
def _build_program():
    import concourse.bacc as bacc
    import concourse.mybir as mybir
    from concourse import tile

    f32 = mybir.dt.float32
    f32r = mybir.dt.float32r
    f16 = mybir.dt.float16
    bf16 = mybir.dt.bfloat16
    AF = mybir.ActivationFunctionType

    nc = bacc.Bacc("TRN2", target_bir_lowering=False, debug=False,
                   num_devices=8)

    di = {}

    def inp(name, shape, dt=f32):
        di[name] = nc.dram_tensor(name, shape, dt, kind="ExternalInput")
        return di[name]

    inp("crit", [128, 9, 544], bf16)   # bdT row 7 ++ fs9a, one DMA
    inp("bdT0", [128, 9, 128], bf16)
    inp("bdT1", [128, 9, 128], bf16)
    inp("bdT26", [128, 5, 9, 128], bf16)
    inp("fs9b", [128, 9, 416], bf16)
    inp("bp", [128, 66, 66], bf16)
    inp("matsb", [128, 9, 128], bf16)
    inp("w1t", [128, 9, 128], bf16)
    inp("w2t", [128, 9, 128], bf16)
    inp("vecs", [128, 6])      # b1, b2, mm4, zc0, zc1, kshv
    inp("onesr", [1, 128], mybir.dt.float32r)
    out_d = nc.dram_tensor("out", [128, 36, 64], bf16,
                           kind="ExternalOutput")

    with tile.TileContext(nc) as tc:
        with tc.tile_pool(name="pers", bufs=1) as pers, \
             tc.tile_pool(name="pps", bufs=1, space="PSUM") as pps:
            # ---------------- persistent tiles + input DMAs ----------------
            bdT = pers.tile([128, 8, 9, 128], bf16, tag="bdT")
            crit = pers.tile([128, 9, 544], bf16, tag="crit")
            fs9b = pers.tile([128, 9, 416], bf16, tag="fs9b")
            bp = pers.tile([128, 66, 66], bf16, tag="bp")
            mats = pers.tile([128, 9, 128], bf16, tag="mats")
            w1t = pers.tile([128, 9, 128], bf16, tag="w1t")
            w2t = pers.tile([128, 9, 128], bf16, tag="w2t")
            vecs = pers.tile([128, 6], f32, tag="vecs")
            onesb = pers.tile([128, 1], bf16, tag="onesb")
            onesr = pers.tile([1, 128], f32r, tag="onesr")
            wdum = pers.tile([128, 512], bf16, tag="wdum")

            # latency-ordered input streaming: every transfer is
            # per-partition contiguous; the two chunks the first score
            # round needs go out on the GpSimd queue, whose prologue
            # completes ~1.7us before Sync's.
            nc.sync.dma_start(crit[:, 0:3], di["crit"].ap()[:, 0:3])
            nc.sync.dma_start(crit[:, 3:6], di["crit"].ap()[:, 3:6])
            nc.sync.dma_start(crit[:, 6:9], di["crit"].ap()[:, 6:9])
            nc.sync.dma_start(fs9b[:], di["fs9b"].ap())
            nc.sync.dma_start(bdT[:, 0], di["bdT0"].ap())
            nc.sync.dma_start(vecs[:], di["vecs"].ap())
            nc.sync.dma_start(bdT[:, 2:7], di["bdT26"].ap())
            nc.sync.dma_start(mats[:], di["matsb"].ap())
            nc.sync.dma_start(bdT[:, 1], di["bdT1"].ap())
            nc.sync.dma_start(bp[:], di["bp"].ap())
            nc.sync.dma_start(w1t[:], di["w1t"].ap())
            nc.sync.dma_start(w2t[:], di["w2t"].ap())
            nc.sync.dma_start(onesr[:], di["onesr"].ap())

            b1v = vecs[:, 0:1]
            b2v = vecs[:, 1:2]
            mm4 = vecs[:, 2:3]
            zc0 = vecs[:, 3:4]
            zc1 = vecs[:, 4:5]
            kshv = vecs[:, 5:6]

            nc.vector.memset(onesb[:], 1.0)
            nc.vector.memset(wdum[:], 0.001)

            S0 = pers.tile([128, 8, WTOT], f32r, tag="S0")
            S1 = pers.tile([128, 8, WTOT], f32r, tag="S1")
            S1h = pers.tile([128, 8, WTOT], bf16, tag="S1h")
            S0h7 = pers.tile([128, WTOT], bf16, tag="S0h7")
            S0h0 = pers.tile([128, WTOT], bf16, tag="S0h0")
            U1s = pers.tile([128, WTOT], f32r, tag="U1s")
            D1s = pers.tile([128, WTOT], f32r, tag="D1s")
            E = pers.tile([128, 8, ND], bf16, tag="E")
            Ssoft = pers.tile([128, 8, ND], bf16, tag="Ssoft")
            R128f = pers.tile([128, ND], f32, tag="R128f")
            R128 = pers.tile([128, ND], bf16, tag="R128")
            den = pers.tile([1, ND], f32r, tag="den")
            img = pers.tile([128, 44, 66], bf16, tag="img")
            img2 = pers.tile([128, 44, 66], bf16, tag="img2")
            outb = pers.tile([128, 36, 64], bf16, tag="outb")
            imgf = img[:].rearrange("p a b -> p (a b)")
            img2f = img2[:].rearrange("p a b -> p (a b)")

            # zero scratch images + the never-written S1 columns that
            # widened fuse2 matmul reads touch (gpsimd, idle now)
            nc.gpsimd.memset(imgf[:, :], 0.0)
            nc.gpsimd.memset(img2f[:, :], 0.0)
            S1f = S1[:].bitcast(f32)
            nc.vector.memset(S1f[:, :, ME:WM], 0.0)
            nc.vector.memset(S1f[:, :, WM + WA - 1:WM + WA], 0.0)
            nc.vector.memset(S1f[:, :, WTOT - 1:WTOT], 0.0)

            # PE warm-up: ~3us of dummy matmuls during the DMA wait so
            # the HAM clock is at 2.4GHz when the scores GEMM starts.
            wps = pps.tile([128, 512], f32, tag="pe", bufs=2)
            for i in range(16):
                nc.tensor.matmul(wps[:, 0:384], wdum[:, 0:128],
                                 wdum[:, 128:512],
                                 start=(i == 0), stop=(i == 15))

            # ---------------- scores GEMM ----------------
            def score_round(r, h, eng=None):
                ps = pps.tile([128, 416], f32, tag="pss", bufs=2)
                for j in range(9):
                    lh = crit[:, j, 0:128] if r == 7 else bdT[:, r, j, :]
                    rh = crit[:, j, 128:544] if h == 0 else fs9b[:, j, :]
                    nc.tensor.matmul(ps[:], lh, rh,
                                     start=(j == 0), stop=(j == 8))
                # evac on DVE by default (faster per-op than scalar and
                # FIFO-ordered with fuse1); the last rounds use scalar so
                # they are never queued behind fuse1 group work
                if eng == "scalar":
                    if h == 0:
                        nc.scalar.activation(S0[:, r, 0:64], ps[:, 0:64],
                                             AF.Copy, bias=0.0, scale=zc0)
                        nc.scalar.copy(S0[:, r, 64:416], ps[:, 64:416])
                    else:
                        nc.scalar.copy(S0[:, r, 416:832], ps[:])
                    return
                if h == 0:
                    nc.vector.tensor_scalar_mul(S0[:, r, 0:64],
                                                ps[:, 0:64], zc0)
                    nc.vector.tensor_copy(S0[:, r, 64:416], ps[:, 64:416])
                else:
                    nc.vector.tensor_copy(S0[:, r, 416:832], ps[:])

            for (r, h) in [(7, 0), (7, 1), (0, 0), (0, 1), (5, 0),
                           (5, 1), (6, 0), (6, 1)]:
                score_round(r, h)
            D1a = pps.tile([128, 416], f32, tag="pss", bufs=2)
            D1b = pps.tile([128, 416], f32, tag="pss", bufs=2)
            U1a = pps.tile([128, 416], f32, tag="pss", bufs=2)
            U1b = pps.tile([128, 416], f32, tag="pss", bufs=2)
            # bf16 copies of rows 7/0 so the shift matmuls run at the
            # fast PE rate (fp32 weights load 3.5x slower and halve the
            # matmul rate)
            nc.scalar.copy(S0h7[:], S0[:, 7, :])
            nc.scalar.copy(S0h0[:], S0[:, 0, :])
            # D1[m] = S0[m-1, 7], U1[m] = S0[m+1, 0] via PE shifts
            nc.tensor.matmul(D1a[:], mats[:, M_D1, :], S0h7[:, 0:416],
                             start=True, stop=True)
            nc.tensor.matmul(D1b[:], mats[:, M_D1, :], S0h7[:, 416:832],
                             start=True, stop=True)
            nc.tensor.matmul(U1a[:], mats[:, M_U1, :], S0h0[:, 0:416],
                             start=True, stop=True)
            nc.tensor.matmul(U1b[:], mats[:, M_U1, :], S0h0[:, 416:832],
                             start=True, stop=True)
            # evacuate wraps to SBUF so fuse2 PSUM never waits on fuse1
            nc.scalar.copy(D1s[:, 0:416], D1a[:])
            nc.scalar.copy(D1s[:, 416:832], D1b[:])
            nc.scalar.copy(U1s[:, 0:416], U1a[:])
            nc.scalar.copy(U1s[:, 416:832], U1b[:])
            for r in (3, 4):
                score_round(r, 0)
                score_round(r, 1)
            for r in (1, 2):
                score_round(r, 0, "scalar")
                score_round(r, 1, "scalar")

            # ------- fuse1 (4 independent groups on DVE / GpSimd) -------
            def fuse1_group(ra, rb, eng):
                """S1[p] = S0[p-1,n-1] + S0[p,n] + S0[p+1,n+1] for rows
                ra..rb-1, main cols [0:ME] + aux windows."""
                ua, ub = ra, min(rb, 7)
                da, db = max(ra, 1), rb
                eng.tensor_add(S1[:, ua:ub, 0:ME],
                               S0[:, ua:ub, 0:ME],
                               S0[:, ua + 1:ub + 1, 1:ME + 1])
                if rb == 8:
                    eng.tensor_add(S1[:, 7, 0:ME], S0[:, 7, 0:ME],
                                   U1s[:, 1:ME + 1])
                eng.tensor_add(S1[:, da:db, 1:ME],
                               S1[:, da:db, 1:ME],
                               S0[:, da - 1:db - 1, 0:ME - 1])
                if ra == 0:
                    eng.tensor_add(S1[:, 0, 1:ME], S1[:, 0, 1:ME],
                                   D1s[:, 0:ME - 1])
                for a0 in (WM, WM + WA):
                    hi = a0 + WA - 1     # last aux col never read
                    eng.tensor_add(S1[:, ua:ub, a0:hi],
                                   S0[:, ua:ub, a0:hi],
                                   S0[:, ua + 1:ub + 1, a0 + 1:a0 + WA])
                    if rb == 8:
                        eng.tensor_add(S1[:, 7, a0:hi], S0[:, 7, a0:hi],
                                       U1s[:, a0 + 1:a0 + WA])
                    eng.tensor_add(S1[:, da:db, a0 + 1:hi],
                                   S1[:, da:db, a0 + 1:hi],
                                   S0[:, da - 1:db - 1, a0:hi - 1])
                    if ra == 0:
                        eng.tensor_add(S1[:, 0, a0 + 1:hi],
                                       S1[:, 0, a0 + 1:hi],
                                       D1s[:, a0:hi - 1])

            # g02|g24 both gate on r2, g46|g68 on r6; pairs run in
            # parallel on the two vector engines, interleaved with the
            # remaining score rounds in each engine's FIFO.
            def fixes(ra, rb):
                # boundary column fixes on the shadow only (S2t's direct
                # term never reads these columns)
                nc.scalar.activation(S1h[:, ra:rb, 63:64],
                                     S1h[:, ra:rb, 63:64],
                                     AF.Copy, bias=0.0, scale=zc0)
                nc.scalar.activation(S1h[:, ra:rb, 640:641],
                                     S1h[:, ra:rb, 640:641],
                                     AF.Copy, bias=0.0, scale=zc1)
                nc.scalar.activation(S1h[:, ra:rb, 735:736],
                                     S1h[:, ra:rb, 735:736],
                                     AF.Copy, bias=0.0, scale=0.0)
                nc.scalar.activation(S1h[:, ra:rb, 800:801],
                                     S1h[:, ra:rb, 800:801],
                                     AF.Copy, bias=0.0, scale=0.0)

            fuse1_group(6, 8, nc.gpsimd)
            fuse1_group(4, 6, nc.vector)
            nc.vector.tensor_scalar_mul(S1h[:, 4:6, :], S1[:, 4:6, :], 1.0)
            nc.vector.tensor_scalar_mul(S1h[:, 6:8, :], S1[:, 6:8, :], 1.0)
            fixes(4, 6)
            fixes(6, 8)
            fuse1_group(2, 4, nc.vector)
            nc.vector.tensor_scalar_mul(S1h[:, 2:4, :], S1[:, 2:4, :], 1.0)
            fixes(2, 4)
            fuse1_group(0, 2, nc.gpsimd)
            # g02's shadow on scalar (the GpSimd convert microcodes to
            # ~14ns/el; DVE is busy with g24's by now)
            nc.scalar.activation(S1h[:, 0:2, :], S1[:, 0:2, :], AF.Copy)
            fixes(0, 2)
            # PE filler so the fuse1 tail never leaves the PE idle past
            # the ~3.4us HAM throttle window
            wps2 = pps.tile([128, 512], f32, tag="pe", bufs=2)
            for i in range(10):
                nc.tensor.matmul(wps2[:, 0:384], wdum[:, 0:128],
                                 wdum[:, 128:512],
                                 start=(i == 0), stop=(i == 9))

            # ---- fuse2: Bp/Bm shifts on PE, base+shift adds on DVE ----
            with tc.tile_pool(name="f2s", bufs=2) as f2s:
                peA = pps.tile([1, 288], f32, tag="pe", bufs=2)
                peB = pps.tile([1, 288], f32, tag="pe", bufs=2)
                prev = []
                for ridx, r in enumerate(R_FUSE2):
                    rp, mp = (r + 1, M_SELP) if r < 7 else (0, M_SELP7)
                    rm, mm_ = (r - 1, M_SELM) if r > 0 else (7, M_SELM0)
                    # all shift terms accumulate in PSUM: Sa covers
                    # S2t[0:416], Sb covers S2t[416:576]; aux wraps are
                    # narrow extra matmuls into the same tiles
                    Sa = pps.tile([128, 416], f32, tag="pd", bufs=2)
                    Sb = pps.tile([128, 416], f32, tag="pu", bufs=2)
                    nc.tensor.matmul(Sa[:, 0:416], mats[:, M_S4P, :],
                                     S1h[:, r, 96:512],
                                     start=True, stop=False)
                    nc.tensor.matmul(Sa[:, 0:416], mats[0:32, mp, :],
                                     S1h[0:32, rp, 96:512],
                                     start=False, stop=False,
                                     skip_group_check=True)
                    nc.tensor.matmul(Sa[:, 0:416], mats[:, M_S4M, :],
                                     S1h[:, r, 32:448],
                                     start=False, stop=False,
                                     skip_group_check=True)
                    nc.tensor.matmul(Sa[:, 0:416], mats[64:128, mm_, :],
                                     S1h[64:128, rm, 32:448],
                                     start=False, stop=False,
                                     skip_group_check=True)
                    nc.tensor.matmul(Sa[:, 1:32], mats[:, M_S4M, :],
                                     S1h[:, r, 736:767],
                                     start=False, stop=False,
                                     skip_group_check=True)
                    nc.tensor.matmul(Sa[:, 1:32], mats[64:128, mm_, :],
                                     S1h[64:128, rm, 736:767],
                                     start=False, stop=True,
                                     skip_group_check=True)
                    nc.tensor.matmul(Sb[:, 0:288], mats[:, M_S4P, :],
                                     S1h[:, r, 512:800],
                                     start=True, stop=False)
                    nc.tensor.matmul(Sb[:, 0:288], mats[0:32, mp, :],
                                     S1h[0:32, rp, 512:800],
                                     start=False, stop=False,
                                     skip_group_check=True)
                    nc.tensor.matmul(Sb[:, 0:320], mats[:, M_S4M, :],
                                     S1h[:, r, 448:768],
                                     start=False, stop=False,
                                     skip_group_check=True)
                    nc.tensor.matmul(Sb[:, 0:320], mats[64:128, mm_, :],
                                     S1h[64:128, rm, 448:768],
                                     start=False, stop=False,
                                     skip_group_check=True)
                    nc.tensor.matmul(Sb[:, 128:159], mats[:, M_S4P, :],
                                     S1h[:, r, 769:800],
                                     start=False, stop=False,
                                     skip_group_check=True)
                    nc.tensor.matmul(Sb[:, 128:159], mats[0:32, mp, :],
                                     S1h[0:32, rp, 769:800],
                                     start=False, stop=True,
                                     skip_group_check=True)
                    # direct term stays f32: S2t = S1[r] + PSUM sums
                    S2t = f2s.tile([128, ND], f32r, tag="S2t")
                    nc.vector.tensor_add(S2t[:, 0:416],
                                         S1[:, r, 64:480],
                                         Sa[:, 0:416])
                    nc.vector.tensor_add(S2t[:, 416:576],
                                         S1[:, r, 480:640],
                                         Sb[:, 0:160])
                    nc.scalar.activation(E[:, r, :], S2t[:], AF.Exp,
                                         bias=kshv, scale=SCALE)
                    # colsum for the previous round (its exp is done;
                    # keeps PE warm without stalling on this round's DVE)
                    if prev:
                        q = prev.pop()
                        nc.tensor.matmul(peA[:], onesb[:, 0:1],
                                         E[:, q, 0:288],
                                         start=(q == R_FUSE2[0]),
                                         stop=False,
                                         skip_group_check=True)
                        nc.tensor.matmul(peB[:], onesb[:, 0:1],
                                         E[:, q, 288:576],
                                         start=(q == R_FUSE2[0]),
                                         stop=False,
                                         skip_group_check=True)
                    prev.append(r)
                q = prev.pop()
                nc.tensor.matmul(peA[:], onesb[:, 0:1], E[:, q, 0:288],
                                 start=False, stop=True,
                                 skip_group_check=True)
                nc.tensor.matmul(peB[:], onesb[:, 0:1], E[:, q, 288:576],
                                 start=False, stop=True,
                                 skip_group_check=True)
                nc.vector.tensor_copy(den[0:1, 0:288], peA[:])
                nc.vector.tensor_copy(den[0:1, 288:576], peB[:])

            # ---------------- softmax via PE reductions ----------------
            for c0 in (0, 288):
                pb = pps.tile([128, 288], f32, tag="pd", bufs=2)
                nc.tensor.matmul(pb[:], onesr[0:1, :],
                                 den[0:1, c0:c0 + 288],
                                 start=True, stop=True)
                nc.vector.reciprocal_approx_fast(R128f[:, c0:c0 + 288],
                                                 pb[:])
            # fold the mm/4 factor into the f32->bf16 cast
            nc.vector.tensor_scalar_mul(R128[:], R128f[:], mm4)
            for r in range(5):
                nc.vector.tensor_mul(Ssoft[:, r, :], E[:, r, :], R128[:])
            for r in range(5, 8):
                nc.gpsimd.tensor_mul(Ssoft[:, r, :], E[:, r, :], R128[:])

            # ---------------- deconv + assembly ----------------
            with tc.tile_pool(name="dc", bufs=2) as dcp:
                for ky in range(4):
                    for kx in range(4):
                        rw = dcp.tile([128, 1024], bf16, tag="rw")
                        nc.scalar.copy(
                            rw[:].rearrange("p (r a b) -> p r a b",
                                            r=8, a=4),
                            bp[:, ky:ky + 63:2, kx:kx + 63:2]
                            .rearrange("p (r a) b -> p r a b", a=4))
                        psA = pps.tile([128, 288], f32, tag="pu", bufs=2)
                        psBt = pps.tile([128, 288], f32, tag="pss", bufs=2)
                        for r in range(8):
                            lh = rw[:, 128 * r:128 * r + 128]
                            nc.tensor.matmul(psA[:], lh, Ssoft[:, r, 0:288],
                                             start=(r == 0), stop=(r == 7))
                            nc.tensor.matmul(psBt[:], lh,
                                             Ssoft[:, r, 288:576],
                                             start=(r == 0), stop=(r == 7))
                        va = img[:, 4 + ky:4 + ky + 18:2, kx:kx + 63:2]
                        vb = img[:, 22 + ky:22 + ky + 18:2, kx:kx + 63:2]
                        nc.vector.tensor_add(
                            va, va, psA[:].rearrange("p (a b) -> p a b",
                                                     b=32))
                        nc.vector.tensor_add(
                            vb, vb, psBt[:].rearrange("p (a b) -> p a b",
                                                      b=32))
            nc.gpsimd.memset(img[:, 4, :], 0.0)
            nc.gpsimd.memset(img[:, 41, :], 0.0)
            nc.gpsimd.memset(img[:, :, 0], 0.0)
            nc.gpsimd.memset(img[:, :, 65], 0.0)

            # ---------------- convs (flat wrap trick) ----------------
            taps3 = [(dy, dx) for dy in range(3) for dx in range(3)]
            for (R, n) in [(4, 7), (11, 7), (18, 7), (25, 7),
                           (32, 7), (39, 3)]:
                L = n * 66 - 2
                ps = pps.tile([128, 462], f32, tag="pe", bufs=2)
                for j, (dy, dx) in enumerate(taps3):
                    base = (R - 1 + dy) * 66 + dx
                    nc.tensor.matmul(ps[:, 0:L], w1t[:, j, :],
                                     imgf[:, base:base + L],
                                     start=(j == 0), stop=(j == 8))
                nc.scalar.activation(
                    img2[:, R:R + n, 1:65],
                    ps[:].rearrange("p (a b) -> p a b", b=66)[:, 0:n,
                                                              0:64],
                    AF.Identity, bias=b1v, scale=1.0)
            nc.gpsimd.memset(img2[:, 4, :], 0.0)
            nc.gpsimd.memset(img2[:, 41, :], 0.0)
            for (R, n) in [(5, 7), (12, 7), (19, 7), (26, 7),
                           (33, 7), (40, 1)]:
                L = n * 66 - 2
                ps = pps.tile([128, 462], f32, tag="pe", bufs=2)
                for j, (dy, dx) in enumerate(taps3):
                    base = (R - 1 + dy) * 66 + dx
                    nc.tensor.matmul(ps[:, 0:L], w2t[:, j, :],
                                     img2f[:, base:base + L],
                                     start=(j == 0), stop=(j == 8))
                nc.scalar.activation(
                    outb[:, R - 5:R - 5 + n, :],
                    ps[:].rearrange("p (a b) -> p a b", b=66)[:, 0:n,
                                                              0:64],
                    AF.Identity, bias=b2v, scale=1.0)
                # stream row-groups out as they complete (the last two
                # go together so the tail is one decent-sized DMA)
                if R <= 26:
                    nc.sync.dma_start(out_d.ap()[:, R - 5:R - 5 + n, :],
                                      outb[:, R - 5:R - 5 + n, :])
            nc.sync.dma_start(out_d.ap()[:, 28:36, :], outb[:, 28:36, :])

    nc.compile()
    return nc


def _get_program():
    if "nc" not in _CACHE:
        _CACHE["nc"] = _build_program()
    return _CACHE["nc"]


# ----------------------------------------------------------------------
# host wrapper
# ----------------------------------------------------------------------
def _build_in_maps(f, b, mask, w1, b1, w2, b2):
    import ml_dtypes
    bf = ml_dtypes.bfloat16

    f = np.asarray(f, np.float32)
    b = np.asarray(b, np.float32)
    mask = np.asarray(mask, np.float32)

    f_ds = _nearest_ds(f, 32, 32)
    b_ds = _nearest_ds(b, 32, 32)
    m_ds = _nearest_ds(mask, 32, 32)
    mp = np.pad(m_ds[0, 0], 1)
    pmean = np.stack([mp[i:i + 32, j:j + 32] for i in range(3)
                      for j in range(3)]).mean()
    mm = np.float32(1.0) if pmean == 0.0 else np.float32(0.0)

    w1t = np.ascontiguousarray(
        np.transpose(np.asarray(w1, np.float32), (1, 2, 3, 0))
        .reshape(128, 9, 128)).astype(bf)
    w2t = np.ascontiguousarray(
        np.transpose(np.asarray(w2, np.float32), (1, 2, 3, 0))
        .reshape(128, 9, 128)).astype(bf)
    consts = {
        "matsb": np.ascontiguousarray(
            _mats().transpose(1, 0, 2)).astype(bf),
        "w1t": w1t, "w2t": w2t,
        "onesr": np.ones((1, 128), np.float32),
    }

    in_maps = []
    for core in range(8):
        bi, h = core // 2, core % 2
        vecs = np.zeros((128, 6), np.float32)
        vecs[:, 0] = np.asarray(b1, np.float32)
        vecs[:, 1] = np.asarray(b2, np.float32)
        vecs[:, 2] = mm / 4.0
        vecs[:, 3] = 0.0 if h == 0 else 1.0
        vecs[:, 4] = 1.0 if h == 0 else 0.0
        vecs[:, 5] = -KSH
        bdT = _make_bdT(b_ds[bi]).astype(bf)          # [128, 9, 1024]
        bdTr = np.ascontiguousarray(
            bdT.reshape(128, 9, 8, 128).transpose(0, 2, 1, 3))
        fs9 = _make_fs9(f_ds[bi], h).astype(bf)       # [128, 9, 832]
        m = dict(consts)
        m.update({
            "crit": np.ascontiguousarray(
                np.concatenate([bdTr[:, 7], fs9[:, :, 0:416]], axis=2)),
            "bdT0": np.ascontiguousarray(bdTr[:, 0]),
            "bdT1": np.ascontiguousarray(bdTr[:, 1]),
            "bdT26": np.ascontiguousarray(bdTr[:, 2:7]),
            "fs9b": np.ascontiguousarray(fs9[:, :, 416:832]),
            "bp": np.ascontiguousarray(
                np.pad(b[bi], ((0, 0), (1, 1), (1, 1)))).astype(bf),
            "vecs": vecs,
        })
        in_maps.append(m)
    return in_maps


def kernel(f, b, mask, w1, b1, w2, b2):
    from concourse.bass_utils import run_bass_kernel_spmd

    in_maps = _build_in_maps(f, b, mask, w1, b1, w2, b2)
    _CACHE["in_maps"] = in_maps
    nc = _get_program()
    res = run_bass_kernel_spmd(nc, in_maps, list(range(8)))

    B, C, H, W = 4, 128, 64, 64
    out = np.empty((B, C, H, W), np.float32)
    for core in range(8):
        bi, h = core // 2, core % 2
        sel = 0 if h == 0 else 4
        out[bi, :, 32 * h:32 * h + 32, :] = \
            res.results[core]["out"][:, sel:sel + 32, :].astype(np.float32)
    return out
